# revision 5
# baseline (speedup 1.0000x reference)
"""Trainium2 Bass kernel for nn_MetaNetLinearizedModel.

Math (B=16, D=12288, F=768, HID=192, T=8):
    X = x.reshape(B, D)
    h1 = X @ W1 + b1                       [B, F]
    g  = gelu_tanh(h1); gp = gelu_tanh'(h1)
    feats = g @ W2 + b2                    [B, F]
    mh = relu(feats @ mW1.T + mb1)         [B, HID]
    coefs = mh @ mW2.T + mb2               [B, T]
    dh1   = sum_t coefs[:,t] * (X @ dW1[t] + db1[t])
    dout  = (gp * dh1) @ W2 + sum_t coefs[:,t] * (g @ dW2[t] + db2[t])
    out   = feats + dout

Sharding over T (8 task vectors -> 8 cores). Core c owns t=c:
    - computes h1 partial from a D-shard of W1 (D/8 rows), AllReduce -> full h1
    - computes U = X @ dW1[c]  (the big 37.7MB/core stream)
    - p_dout[c] = (gp * coefs[:,c]*(U + db1[c])) @ W2
                  + coefs[:,c] * (g @ dW2[c] + db2[c])
      (the coefs[:,c] scale commutes; applied once at the end)
    - host: out = feats + sum_c p_dout[c]

On-chip layout is transposed: everything is [F-on-partitions, B-on-free]
so matmuls contract over partitions with M=128 (full PE array), N=16.
"""
import sys

sys.path.insert(0, "/opt/trn_rl_repo")

import numpy as np
import concourse.bass as bass
import concourse.bacc as bacc
import concourse.tile as tile
import concourse.mybir as mybir
from concourse import bass_utils

F32 = mybir.dt.float32
AF = mybir.ActivationFunctionType
OP = mybir.AluOpType

B, Cc, Hh, Ww = 16, 3, 64, 64
D = Cc * Hh * Ww * 1  # 12288... (C*H*W)
D = 3 * 64 * 64
F = 768
HID = 192
T = 8
NCORES = 8
DSH = D // NCORES      # 1536
KD = D // 128          # 96 k-tiles over D
KJ = KD // 2           # 48 double-tiles
KSH = DSH // 128       # 12
FO = F // 128          # 6
KF = F // 128          # 6
GELU_C0 = float(np.sqrt(2.0 / np.pi))
GELU_A = 0.044715

_CACHE = {}


def build():
    nc = bacc.Bacc("TRN2", target_bir_lowering=False, debug=False,
                   enable_asserts=False, num_devices=NCORES)

    XT = nc.dram_tensor("xt", [D, B], F32, kind="ExternalInput")
    XTS = nc.dram_tensor("xts", [DSH, B], F32, kind="ExternalInput")
    W1S = nc.dram_tensor("w1s", [DSH, F], F32, kind="ExternalInput")
    W2 = nc.dram_tensor("w2", [F, F], F32, kind="ExternalInput")
    DW1 = nc.dram_tensor("dw1", [D, F], F32, kind="ExternalInput")
    DW2 = nc.dram_tensor("dw2", [F, F], F32, kind="ExternalInput")
    MW1T = nc.dram_tensor("mw1t", [F, HID], F32, kind="ExternalInput")
    MW2T = nc.dram_tensor("mw2t", [HID, T], F32, kind="ExternalInput")
    BIAS = nc.dram_tensor("bias", [128, 28], F32, kind="ExternalInput")
    SEL = nc.dram_tensor("sel", [T, 128], F32, kind="ExternalInput")
    OT = nc.dram_tensor("ot", [128, FO * B], F32, kind="ExternalOutput")
    FT = nc.dram_tensor("ft", [128, FO * B], F32, kind="ExternalOutput")

    # bias_pack columns
    BC_B1, BC_B2, BC_DB1, BC_DB2, BC_MB1, BC_MB2 = 0, 6, 12, 18, 24, 26

    with tile.TileContext(nc, num_cores=NCORES) as tc:
        with (
            tc.tile_pool(name="cst", bufs=1) as cst,
            tc.tile_pool(name="wrk", bufs=1) as wrk,
            tc.tile_pool(name="dw1p", bufs=12) as dw1p,
            tc.tile_pool(name="psu", bufs=1, space="PSUM") as psu,
            tc.tile_pool(name="pss", bufs=2, space="PSUM") as pss,
            tc.tile_pool(name="drm", bufs=1, space="DRAM") as drm,
        ):
            # ---------------- phase B: loads on ACT ring ----------------
            xt_sb = cst.tile([128, KD * B], F32)      # XT packed [p, (d-tile, b)]
            nc.scalar.dma_start(
                xt_sb[:].rearrange("p (n b) -> p n b", b=B),
                XT.ap().rearrange("(n p) b -> p n b", p=128))
            xts_sb = cst.tile([128, KSH * B], F32)
            nc.scalar.dma_start(
                xts_sb[:].rearrange("p (n b) -> p n b", b=B),
                XTS.ap().rearrange("(n p) b -> p n b", p=128))
            w1s_sb = cst.tile([128, KSH * F], F32)    # [p, (k-tile, f)]
            for k in range(KSH):
                nc.scalar.dma_start(
                    w1s_sb[:, k * F:(k + 1) * F],
                    W1S.ap()[k * 128:(k + 1) * 128, :])
            w2_sb = cst.tile([128, KF * F], F32)
            for k in range(KF):
                nc.scalar.dma_start(
                    w2_sb[:, k * F:(k + 1) * F],
                    W2.ap()[k * 128:(k + 1) * 128, :])
            dw2_sb = cst.tile([128, KF * F], F32)
            for k in range(KF):
                nc.scalar.dma_start(
                    dw2_sb[:, k * F:(k + 1) * F],
                    DW2.ap()[k * 128:(k + 1) * 128, :])
            mw1t_sb = cst.tile([128, KF * HID], F32)
            for k in range(KF):
                nc.scalar.dma_start(
                    mw1t_sb[:, k * HID:(k + 1) * HID],
                    MW1T.ap()[k * 128:(k + 1) * 128, :])
            mw2t_sb = cst.tile([128, 2 * T], F32)
            nc.scalar.dma_start(mw2t_sb[:, 0:T], MW2T.ap()[0:128, :])
            nc.scalar.dma_start(mw2t_sb[0:HID - 128, T:2 * T],
                                MW2T.ap()[128:HID, :])
            bias_sb = cst.tile([128, 28], F32)
            nc.scalar.dma_start(bias_sb[:], BIAS.ap())
            sel_sb = cst.tile([T, 128], F32)
            nc.scalar.dma_start(sel_sb[:], SEL.ap())

            # ---------------- h1 partial + AllReduce ----------------
            h1p_sb = wrk.tile([128, FO * B], F32)
            for fo in range(FO):
                ps = pss.tile([128, B], F32, name="ps", tag="ps")
                for k in range(KSH):
                    nc.tensor.matmul(
                        ps[:],
                        w1s_sb[:, k * F + fo * 128:k * F + (fo + 1) * 128],
                        xts_sb[:, k * B:(k + 1) * B],
                        start=(k == 0), stop=(k == KSH - 1))
                nc.vector.tensor_copy(h1p_sb[:, fo * B:(fo + 1) * B], ps[:])
            h1p_d = drm.tile([128, FO * B], F32)
            h1ar_d = drm.tile([128, FO * B], F32)
            nc.gpsimd.dma_start(h1p_d[:], h1p_sb[:])
            nc.gpsimd.collective_compute(
                "AllReduce", OP.add,
                replica_groups=[list(range(NCORES))],
                ins=[h1p_d[:]], outs=[h1ar_d[:]])
            h1r_sb = wrk.tile([128, FO * B], F32)
            nc.gpsimd.dma_start(h1r_sb[:], h1ar_d[:])

            # h1 = h1r + b1  (per-partition bias per f-tile)
            h1_sb = wrk.tile([128, FO * B], F32)
            for fo in range(FO):
                nc.scalar.activation(
                    h1_sb[:, fo * B:(fo + 1) * B],
                    h1r_sb[:, fo * B:(fo + 1) * B],
                    AF.Identity, bias=bias_sb[:, BC_B1 + fo:BC_B1 + fo + 1])

            # ---------------- gelu(h1), gelu'(h1) ----------------
            W = FO * B
            s_sb = wrk.tile([128, W], F32)
            nc.vector.tensor_mul(s_sb[:], h1_sb[:], h1_sb[:])
            t1_sb = wrk.tile([128, W], F32)
            nc.vector.scalar_tensor_tensor(
                t1_sb[:], s_sb[:], GELU_A, h1_sb[:], OP.mult, OP.mult)
            up_sb = wrk.tile([128, W], F32)
            nc.vector.tensor_add(up_sb[:], h1_sb[:], t1_sb[:])
            t_sb = wrk.tile([128, W], F32)
            nc.scalar.activation(t_sb[:], up_sb[:], AF.Tanh, scale=GELU_C0)
            t2_sb = wrk.tile([128, W], F32)
            nc.vector.tensor_mul(t2_sb[:], t_sb[:], t_sb[:])
            q_sb = wrk.tile([128, W], F32)
            nc.vector.tensor_scalar(q_sb[:], t2_sb[:], -1.0, 1.0, OP.mult, OP.add)
            r_sb = wrk.tile([128, W], F32)
            nc.vector.tensor_scalar(r_sb[:], s_sb[:], 3.0 * GELU_A, 1.0,
                                    OP.mult, OP.add)
            m_sb = wrk.tile([128, W], F32)
            nc.vector.tensor_mul(m_sb[:], q_sb[:], r_sb[:])
            n_sb = wrk.tile([128, W], F32)
            nc.vector.tensor_mul(n_sb[:], m_sb[:], h1_sb[:])
            th_sb = wrk.tile([128, W], F32)
            nc.vector.tensor_scalar(th_sb[:], t_sb[:], 0.5, 0.5, OP.mult, OP.add)
            gp_sb = wrk.tile([128, W], F32)
            nc.vector.scalar_tensor_tensor(
                gp_sb[:], n_sb[:], 0.5 * GELU_C0, th_sb[:], OP.mult, OP.add)
            g_sb = wrk.tile([128, W], F32)
            nc.vector.tensor_mul(g_sb[:], th_sb[:], h1_sb[:])

            # ---------------- feats = g @ W2 + b2 ----------------
            feats_sb = wrk.tile([128, W], F32)
            for fo in range(FO):
                ps = pss.tile([128, B], F32, name="ps", tag="ps")
                for k in range(KF):
                    nc.tensor.matmul(
                        ps[:],
                        w2_sb[:, k * F + fo * 128:k * F + (fo + 1) * 128],
                        g_sb[:, k * B:(k + 1) * B],
                        start=(k == 0), stop=(k == KF - 1))
                nc.scalar.activation(
                    feats_sb[:, fo * B:(fo + 1) * B], ps[:],
                    AF.Identity, bias=bias_sb[:, BC_B2 + fo:BC_B2 + fo + 1])
            nc.scalar.dma_start(FT.ap(), feats_sb[:])

            # ---------------- meta-net ----------------
            mh_sb = wrk.tile([128, 2 * B], F32)
            for hm, msz in ((0, 128), (1, HID - 128)):
                ps = pss.tile([128, B], F32, name="ps", tag="ps")
                for k in range(KF):
                    nc.tensor.matmul(
                        ps[0:msz, :],
                        mw1t_sb[:, k * HID + hm * 128:k * HID + hm * 128 + msz],
                        feats_sb[:, k * B:(k + 1) * B],
                        start=(k == 0), stop=(k == KF - 1))
                nc.scalar.activation(
                    mh_sb[0:msz, hm * B:(hm + 1) * B], ps[0:msz, :],
                    AF.Relu, bias=bias_sb[0:msz, BC_MB1 + hm:BC_MB1 + hm + 1])
            coefs_sb = wrk.tile([T, B], F32)
            ps = pss.tile([128, B], F32, name="ps", tag="ps")
            nc.tensor.matmul(ps[0:T, :], mw2t_sb[0:128, 0:T], mh_sb[:, 0:B],
                             start=True, stop=False)
            nc.tensor.matmul(ps[0:T, :], mw2t_sb[0:HID - 128, T:2 * T],
                             mh_sb[0:HID - 128, B:2 * B],
                             start=False, stop=True)
            nc.scalar.activation(coefs_sb[:], ps[0:T, :], AF.Identity,
                                 bias=bias_sb[0:T, BC_MB2:BC_MB2 + 1])
            # broadcast this core's coef row to all 128 partitions
            cbc_sb = wrk.tile([128, B], F32)
            ps = pss.tile([128, B], F32, name="ps", tag="ps")
            nc.tensor.matmul(ps[:], sel_sb[:, :], coefs_sb[:], start=True,
                             stop=True)
            nc.vector.tensor_copy(cbc_sb[:], ps[:])

            # ---------------- V = g @ dW2 + db2 ----------------
            v_sb = wrk.tile([128, W], F32)
            for fo in range(FO):
                ps = pss.tile([128, B], F32, name="ps", tag="ps")
                for k in range(KF):
                    nc.tensor.matmul(
                        ps[:],
                        dw2_sb[:, k * F + fo * 128:k * F + (fo + 1) * 128],
                        g_sb[:, k * B:(k + 1) * B],
                        start=(k == 0), stop=(k == KF - 1))
                nc.scalar.activation(
                    v_sb[:, fo * B:(fo + 1) * B], ps[:],
                    AF.Identity, bias=bias_sb[:, BC_DB2 + fo:BC_DB2 + fo + 1])

            # ---------------- U = X @ dW1 (the big stream) ----------------
            u_ps = [psu.tile([128, B], F32, name=f"u{fo}") for fo in range(FO)]
            dw1_r = DW1.ap().rearrange("(n two p) f -> n p two f", two=2,
                                       p=128)
            for j in range(KJ):
                dt_ = dw1p.tile([128, 2 * F], F32, name="dw1t")
                nc.sync.dma_start(
                    dt_[:].rearrange("p (two f) -> p two f", two=2), dw1_r[j])
                for half in range(2):
                    d = 2 * j + half
                    for fo in range(FO):
                        nc.tensor.matmul(
                            u_ps[fo][:],
                            dt_[:, half * F + fo * 128:half * F + (fo + 1) * 128],
                            xt_sb[:, d * B:(d + 1) * B],
                            start=(d == 0), stop=(d == KD - 1),
                            skip_group_check=True)

            # ---------------- tail ----------------
            z1_sb = wrk.tile([128, W], F32)
            for fo in range(FO):
                # z1 = (U + db1) * gp
                nc.vector.scalar_tensor_tensor(
                    z1_sb[:, fo * B:(fo + 1) * B],
                    u_ps[fo][:],
                    bias_sb[:, BC_DB1 + fo:BC_DB1 + fo + 1],
                    gp_sb[:, fo * B:(fo + 1) * B],
                    OP.add, OP.mult)
            out_sb = wrk.tile([128, W], F32)
            for fo in range(FO):
                ps = pss.tile([128, B], F32, name="ps", tag="ps")
                for k in range(KF):
                    nc.tensor.matmul(
                        ps[:],
                        w2_sb[:, k * F + fo * 128:k * F + (fo + 1) * 128],
                        z1_sb[:, k * B:(k + 1) * B],
                        start=(k == 0), stop=(k == KF - 1))
                # out = (z2 + v) * cbc
                tmp_sb = wrk.tile([128, B], F32, name="tmp_o")
                nc.vector.tensor_add(tmp_sb[:], ps[:],
                                     v_sb[:, fo * B:(fo + 1) * B])
                nc.vector.tensor_mul(out_sb[:, fo * B:(fo + 1) * B],
                                     tmp_sb[:], cbc_sb[:])
            nc.scalar.dma_start(OT.ap(), out_sb[:])

    nc.compile()
    return nc


def _get_nc():
    if "nc" not in _CACHE:
        _CACHE["nc"] = build()
    return _CACHE["nc"]


def _prep_in_maps(x, W1, b1, W2, b2, mW1, mb1, mW2, mb2, dW1, db1, dW2, db2):
    f32 = np.float32
    X = np.ascontiguousarray(x.reshape(B, D), dtype=f32)
    XT = np.ascontiguousarray(X.T)
    W1 = np.asarray(W1, dtype=f32)
    W2c = np.ascontiguousarray(W2, dtype=f32)
    mw1t = np.ascontiguousarray(np.asarray(mW1, f32).T)
    mw2t = np.ascontiguousarray(np.asarray(mW2, f32).T)
    b1 = np.asarray(b1, f32); b2 = np.asarray(b2, f32)
    mb1 = np.asarray(mb1, f32); mb2 = np.asarray(mb2, f32)
    dW1 = np.asarray(dW1, f32); db1 = np.asarray(db1, f32)
    dW2 = np.asarray(dW2, f32); db2 = np.asarray(db2, f32)

    in_maps = []
    for c in range(NCORES):
        bias = np.zeros((128, 28), f32)
        bias[:, 0:6] = b1.reshape(6, 128).T
        bias[:, 6:12] = b2.reshape(6, 128).T
        bias[:, 12:18] = db1[c].reshape(6, 128).T
        bias[:, 18:24] = db2[c].reshape(6, 128).T
        bias[:, 24:26] = np.pad(mb1, (0, 256 - HID)).reshape(2, 128).T
        bias[0:T, 26] = mb2
        sel = np.zeros((T, 128), f32)
        sel[c, :] = 1.0
        in_maps.append({
            "xt": XT,
            "xts": np.ascontiguousarray(XT[c * DSH:(c + 1) * DSH, :]),
            "w1s": np.ascontiguousarray(W1[c * DSH:(c + 1) * DSH, :]),
            "w2": W2c,
            "dw1": np.ascontiguousarray(dW1[c]),
            "dw2": np.ascontiguousarray(dW2[c]),
            "mw1t": mw1t,
            "mw2t": mw2t,
            "bias": bias,
            "sel": sel,
        })
    return in_maps


def _unpack(mat):
    # [128, 6*16] -> [16, 768]:  out[b, fo*128+p] = mat[p, fo*16+b]
    return np.ascontiguousarray(
        mat.reshape(128, FO, B).transpose(2, 1, 0).reshape(B, F))


def run(inputs, trace=False, trace_cores=None, tmpdir=None):
    nc = _get_nc()
    in_maps = _prep_in_maps(**inputs)
    res = bass_utils.run_bass_kernel_spmd(
        nc, in_maps, core_ids=list(range(NCORES)), trace=trace,
        trace_cores=trace_cores, tmpdir=tmpdir)
    feats = _unpack(res.results[0]["ft"]).astype(np.float64)
    acc = feats
    for c in range(NCORES):
        acc = acc + _unpack(res.results[c]["ot"]).astype(np.float64)
    out = acc.astype(np.float32)
    return out, res


def kernel(**inputs):
    out, _ = run(inputs, trace=False)
    return out


# revision 8
# speedup vs baseline: 1.5701x; 1.5701x over previous
"""Trainium2 Bass kernel for nn_MetaNetLinearizedModel.

Math (B=16, D=12288, F=768, HID=192, T=8):
    X = x.reshape(B, D)
    h1 = X @ W1 + b1                       [B, F]
    g  = gelu_tanh(h1); gp = gelu_tanh'(h1)
    feats = g @ W2 + b2                    [B, F]
    mh = relu(feats @ mW1.T + mb1)         [B, HID]
    coefs = mh @ mW2.T + mb2               [B, T]
    dh1   = sum_t coefs[:,t] * (X @ dW1[t] + db1[t])
    dout  = (gp * dh1) @ W2 + sum_t coefs[:,t] * (g @ dW2[t] + db2[t])
    out   = feats + dout

Sharding over T (8 task vectors -> 8 cores). Core c owns t=c:
    - h1 partial from a D-shard of W1 (D/8 rows), AllReduce -> full h1
    - U = X @ dW1[c]  (the big 37.7MB/core stream)
    - p_dout[c] = (gp * (U + db1[c])) @ W2 + (g @ dW2[c] + db2[c]),
      all scaled by coefs[:,c] at the end (the scale commutes)
    - host: out = feats + sum_c p_dout[c]

Orientation: outputs live in [B(=16, padded 32) partitions, F free] layout.
Big tensors (dW1, W1-shard, W2, dW2, mW1T, mW2T) are MOVING operands
(N=512/256 free), the small activation [d|f, b] tiles are stationary.
Bulk/err-tolerant paths use float32r (full PE rate); h1/feats/Z2 stay fp32.
"""
import sys

sys.path.insert(0, "/opt/trn_rl_repo")

import numpy as np
import concourse.bass as bass
import concourse.bacc as bacc
import concourse.tile as tile
import concourse.mybir as mybir
from concourse import bass_utils

F32 = mybir.dt.float32
F32R = mybir.dt.float32r
AF = mybir.ActivationFunctionType
OP = mybir.AluOpType

B = 16
D = 3 * 64 * 64        # 12288
F = 768
HID = 192
T = 8
NCORES = 8
DSH = D // NCORES      # 1536
KD = D // 128          # 96 k-tiles over D
KJ = KD // 2           # 48 double-tiles
KSH = DSH // 128       # 12
FO = F // 128          # 6
KF = F // 128          # 6
GELU_C0 = float(np.sqrt(2.0 / np.pi))
GELU_A = 0.044715

# bc pack column offsets
BC_B1, BC_B2, BC_DB1, BC_DB2 = 0, F, 2 * F, 3 * F
BC_MB1 = 4 * F                 # width HID
BC_MB2 = BC_MB1 + HID          # width T
BC_SEL = BC_MB2 + T            # width T
BC_EYE = BC_SEL + T            # width 32
BC_W = BC_EYE + 32             # 3312

_CACHE = {}


def build():
    nc = bacc.Bacc("TRN2", target_bir_lowering=False, debug=False,
                   enable_asserts=False, num_devices=NCORES)

    XT = nc.dram_tensor("xt", [D, B], F32R, kind="ExternalInput")
    XTS = nc.dram_tensor("xts", [DSH, B], F32, kind="ExternalInput")
    W1S = nc.dram_tensor("w1s", [DSH, F], F32, kind="ExternalInput")
    W2 = nc.dram_tensor("w2", [F, F], F32, kind="ExternalInput")
    DW1 = nc.dram_tensor("dw1", [D, F], F32R, kind="ExternalInput")
    DW2 = nc.dram_tensor("dw2", [F, F], F32R, kind="ExternalInput")
    MW1T = nc.dram_tensor("mw1t", [F, HID], F32R, kind="ExternalInput")
    MW2T = nc.dram_tensor("mw2t", [HID, T], F32R, kind="ExternalInput")
    BC = nc.dram_tensor("bc", [32, BC_W], F32, kind="ExternalInput")
    OT = nc.dram_tensor("ot", [B, F], F32, kind="ExternalOutput")
    FT = nc.dram_tensor("ft", [B, F], F32, kind="ExternalOutput")

    with tile.TileContext(nc, num_cores=NCORES) as tc:
        with (
            tc.tile_pool(name="cst", bufs=1) as cst,
            tc.tile_pool(name="wrk", bufs=1) as wrk,
            tc.tile_pool(name="gtmp", bufs=4) as gtmp,
            tc.tile_pool(name="w1sp", bufs=4) as w1sp,
            tc.tile_pool(name="dw1p", bufs=8) as dw1p,
            tc.tile_pool(name="psu", bufs=1, space="PSUM") as psu,
            tc.tile_pool(name="pss", bufs=4, space="PSUM") as pss,
            tc.tile_pool(name="drm", bufs=1, space="DRAM") as drm,
        ):
            # ------------- loads (ACT HWDGE ring) -------------
            xt_sb = cst.tile([128, KD * B], F32R)
            nc.scalar.dma_start(
                xt_sb[:].rearrange("p (n b) -> p n b", b=B),
                XT.ap().rearrange("(n p) b -> p n b", p=128))
            xts_sb = cst.tile([128, KSH * B], F32)
            nc.scalar.dma_start(
                xts_sb[:].rearrange("p (n b) -> p n b", b=B),
                XTS.ap().rearrange("(n p) b -> p n b", p=128))
            bc_sb = cst.tile([32, BC_W], F32)
            nc.scalar.dma_start(bc_sb[:], BC.ap())
            w2_sb = cst.tile([128, KF * F], F32)
            for k in range(KF):
                nc.scalar.dma_start(
                    w2_sb[:, k * F:(k + 1) * F],
                    W2.ap()[k * 128:(k + 1) * 128, :])
            dw2_sb = cst.tile([128, KF * F], F32R)
            for k in range(KF):
                nc.scalar.dma_start(
                    dw2_sb[:, k * F:(k + 1) * F],
                    DW2.ap()[k * 128:(k + 1) * 128, :])
            mw1t_sb = cst.tile([128, KF * HID], F32R)
            for k in range(KF):
                nc.scalar.dma_start(
                    mw1t_sb[:, k * HID:(k + 1) * HID],
                    MW1T.ap()[k * 128:(k + 1) * 128, :])
            mw2t_sb = cst.tile([128, 2 * T], F32R)
            nc.scalar.dma_start(mw2t_sb[:, 0:T], MW2T.ap()[0:128, :])
            nc.scalar.dma_start(mw2t_sb[0:HID - 128, T:2 * T],
                                MW2T.ap()[128:HID, :])

            def bcs(col, w):
                return bc_sb[0:B, col:col + w]
            eye = bc_sb[0:32, BC_EYE:BC_EYE + 32]

            # ------------- h1 partial (fp32) + AllReduce -------------
            h5 = pss.tile([B, 512], F32, name="ps", tag="ps")
            h2 = pss.tile([B, 256], F32, name="ps", tag="ps")
            for k in range(KSH):
                wt = w1sp.tile([128, F], F32, name="w1t", tag="w1t")
                nc.scalar.dma_start(wt[:], W1S.ap()[k * 128:(k + 1) * 128, :])
                nc.tensor.matmul(h5[:], xts_sb[:, k * B:(k + 1) * B],
                                 wt[:, 0:512],
                                 start=(k == 0), stop=(k == KSH - 1))
                nc.tensor.matmul(h2[:], xts_sb[:, k * B:(k + 1) * B],
                                 wt[:, 512:F],
                                 start=(k == 0), stop=(k == KSH - 1))
            h1p_sb = wrk.tile([32, F], F32)
            nc.vector.tensor_copy(h1p_sb[0:B, 0:512], h5[:])
            nc.vector.tensor_copy(h1p_sb[0:B, 512:F], h2[:])
            h1p_d = drm.tile([B, F], F32)
            h1ar_d = drm.tile([B, F], F32)
            nc.gpsimd.dma_start(h1p_d[:], h1p_sb[0:B, :])
            nc.gpsimd.collective_compute(
                "AllReduce", OP.add,
                replica_groups=[list(range(NCORES))],
                ins=[h1p_d[:]], outs=[h1ar_d[:]])
            h1r_sb = wrk.tile([32, F], F32)
            nc.gpsimd.dma_start(h1r_sb[0:B, :], h1ar_d[:])
            h1_bt = wrk.tile([32, F], F32)
            nc.vector.tensor_add(h1_bt[0:B, :], h1r_sb[0:B, :],
                                 bcs(BC_B1, F))

            # ------------- gelu(h1), gelu'(h1)  (rows 0:B) -------------
            def gt():
                return gtmp.tile([32, F], F32, name="gt", tag="gt")
            h1 = h1_bt[0:B, :]
            s_ = gt(); nc.vector.tensor_mul(s_[0:B, :], h1, h1)
            t1 = gt(); nc.vector.scalar_tensor_tensor(
                t1[0:B, :], s_[0:B, :], GELU_A, h1, OP.mult, OP.mult)
            up = gt(); nc.vector.tensor_add(up[0:B, :], h1, t1[0:B, :])
            t_sb = wrk.tile([32, F], F32)
            nc.scalar.activation(t_sb[0:B, :], up[0:B, :], AF.Tanh,
                                 scale=GELU_C0)
            tt = t_sb[0:B, :]
            t2 = gt(); nc.vector.tensor_mul(t2[0:B, :], tt, tt)
            q_ = gt(); nc.vector.tensor_scalar(q_[0:B, :], t2[0:B, :],
                                               -1.0, 1.0, OP.mult, OP.add)
            r_ = gt(); nc.vector.tensor_scalar(r_[0:B, :], s_[0:B, :],
                                               3.0 * GELU_A, 1.0,
                                               OP.mult, OP.add)
            m_ = gt(); nc.vector.tensor_mul(m_[0:B, :], q_[0:B, :], r_[0:B, :])
            n_ = gt(); nc.vector.tensor_mul(n_[0:B, :], m_[0:B, :], h1)
            th_sb = wrk.tile([32, F], F32)
            nc.vector.tensor_scalar(th_sb[0:B, :], tt, 0.5, 0.5,
                                    OP.mult, OP.add)
            gp_bt = wrk.tile([32, F], F32)
            nc.vector.scalar_tensor_tensor(
                gp_bt[0:B, :], n_[0:B, :], 0.5 * GELU_C0, th_sb[0:B, :],
                OP.mult, OP.add)
            g_bt = wrk.tile([32, F], F32)
            nc.vector.memset(g_bt[:], 0.0)
            nc.vector.tensor_mul(g_bt[0:B, :], th_sb[0:B, :], h1)

            # ------------- transpose g -> [f, b] stationary tiles -------------
            g_t = wrk.tile([128, FO * B], F32)
            g_tr = wrk.tile([128, FO * B], F32R)
            for fo in range(FO):
                tp = pss.tile([128, 32], F32, name="ps", tag="ps")
                nc.tensor.transpose(tp[:], g_bt[0:32, fo * 128:(fo + 1) * 128],
                                    eye)
                nc.vector.tensor_copy(g_t[:, fo * B:(fo + 1) * B], tp[:, 0:B])
                nc.vector.tensor_copy(g_tr[:, fo * B:(fo + 1) * B], tp[:, 0:B])

            # ------------- feats = g @ W2 + b2 (fp32) -------------
            f5 = pss.tile([B, 512], F32, name="ps", tag="ps")
            f2 = pss.tile([B, 256], F32, name="ps", tag="ps")
            for k in range(KF):
                nc.tensor.matmul(f5[:], g_t[:, k * B:(k + 1) * B],
                                 w2_sb[:, k * F:k * F + 512],
                                 start=(k == 0), stop=(k == KF - 1))
                nc.tensor.matmul(f2[:], g_t[:, k * B:(k + 1) * B],
                                 w2_sb[:, k * F + 512:(k + 1) * F],
                                 start=(k == 0), stop=(k == KF - 1))
            feats_bt = wrk.tile([32, F], F32)
            nc.vector.memset(feats_bt[:], 0.0)
            nc.vector.tensor_add(feats_bt[0:B, 0:512], f5[:], bcs(BC_B2, 512))
            nc.vector.tensor_add(feats_bt[0:B, 512:F], f2[:],
                                 bc_sb[0:B, BC_B2 + 512:BC_B2 + F])
            nc.scalar.dma_start(FT.ap(), feats_bt[0:B, :])

            feats_tr = wrk.tile([128, FO * B], F32R)
            for fo in range(FO):
                tp = pss.tile([128, 32], F32, name="ps", tag="ps")
                nc.tensor.transpose(tp[:],
                                    feats_bt[0:32, fo * 128:(fo + 1) * 128],
                                    eye)
                nc.vector.tensor_copy(feats_tr[:, fo * B:(fo + 1) * B],
                                      tp[:, 0:B])

            # ------------- meta-net (fp32r) -------------
            mps = pss.tile([B, HID], F32, name="ps", tag="ps")
            for k in range(KF):
                nc.tensor.matmul(mps[:], feats_tr[:, k * B:(k + 1) * B],
                                 mw1t_sb[:, k * HID:(k + 1) * HID],
                                 start=(k == 0), stop=(k == KF - 1))
            mh_bt = wrk.tile([32, HID], F32)
            nc.vector.memset(mh_bt[:], 0.0)
            mtmp = gtmp.tile([32, F], F32, name="gt", tag="gt")
            nc.vector.tensor_add(mtmp[0:B, 0:HID], mps[:], bcs(BC_MB1, HID))
            nc.vector.tensor_relu(mh_bt[0:B, :], mtmp[0:B, 0:HID])

            mh_tr = wrk.tile([128, 2 * B], F32R)
            tp = pss.tile([128, 32], F32, name="ps", tag="ps")
            nc.tensor.transpose(tp[:], mh_bt[0:32, 0:128], eye)
            nc.vector.tensor_copy(mh_tr[:, 0:B], tp[:, 0:B])
            tp = pss.tile([128, 32], F32, name="ps", tag="ps")
            nc.tensor.transpose(tp[0:HID - 128, :], mh_bt[0:32, 128:HID], eye)
            nc.vector.tensor_copy(mh_tr[0:HID - 128, B:2 * B],
                                  tp[0:HID - 128, 0:B])

            cps = pss.tile([B, T], F32, name="ps", tag="ps")
            nc.tensor.matmul(cps[:], mh_tr[:, 0:B], mw2t_sb[:, 0:T],
                             start=True, stop=False)
            nc.tensor.matmul(cps[:], mh_tr[0:HID - 128, B:2 * B],
                             mw2t_sb[0:HID - 128, T:2 * T],
                             start=False, stop=True)
            coefs_bt = wrk.tile([B, T], F32)
            nc.vector.tensor_add(coefs_bt[:], cps[:], bcs(BC_MB2, T))
            csel = wrk.tile([B, 1], F32)
            cjunk = wrk.tile([B, T], F32)
            nc.vector.tensor_mul(cjunk[:], coefs_bt[:], bcs(BC_SEL, T))
            nc.vector.reduce_sum(csel[:], cjunk[:], axis=mybir.AxisListType.X)

            # ------------- V = g @ dW2 + db2 (fp32r) -------------
            v5 = pss.tile([B, 512], F32, name="ps", tag="ps")
            v2 = pss.tile([B, 256], F32, name="ps", tag="ps")
            for k in range(KF):
                nc.tensor.matmul(v5[:], g_tr[:, k * B:(k + 1) * B],
                                 dw2_sb[:, k * F:k * F + 512],
                                 start=(k == 0), stop=(k == KF - 1))
                nc.tensor.matmul(v2[:], g_tr[:, k * B:(k + 1) * B],
                                 dw2_sb[:, k * F + 512:(k + 1) * F],
                                 start=(k == 0), stop=(k == KF - 1))
            v_bt = wrk.tile([32, F], F32)
            nc.vector.tensor_add(v_bt[0:B, 0:512], v5[:], bcs(BC_DB2, 512))
            nc.vector.tensor_add(v_bt[0:B, 512:F], v2[:],
                                 bc_sb[0:B, BC_DB2 + 512:BC_DB2 + F])

            # ------------- U = X @ dW1 (fp32r big stream, SP ring) -------------
            u5 = psu.tile([B, 512], F32, name="u5")
            u2 = psu.tile([B, 256], F32, name="u2")
            dw1_r = DW1.ap().rearrange("(n two p) f -> n p two f", two=2,
                                       p=128)
            for j in range(KJ):
                dt_ = dw1p.tile([128, 2 * F], F32R, name="dw1t", tag="dw1t")
                nc.sync.dma_start(
                    dt_[:].rearrange("p (two f) -> p two f", two=2), dw1_r[j])
                for half in range(2):
                    d = 2 * j + half
                    nc.tensor.matmul(
                        u5[:], xt_sb[:, d * B:(d + 1) * B],
                        dt_[:, half * F:half * F + 512],
                        start=(d == 0), stop=(d == KD - 1),
                        skip_group_check=True)
                    nc.tensor.matmul(
                        u2[:], xt_sb[:, d * B:(d + 1) * B],
                        dt_[:, half * F + 512:(half + 1) * F],
                        start=(d == 0), stop=(d == KD - 1),
                        skip_group_check=True)

            # ------------- tail -------------
            z1_bt = wrk.tile([32, F], F32)
            nc.vector.memset(z1_bt[:], 0.0)
            tz = gtmp.tile([32, F], F32, name="gt", tag="gt")
            nc.vector.tensor_add(tz[0:B, 0:512], u5[:], bcs(BC_DB1, 512))
            nc.vector.tensor_add(tz[0:B, 512:F], u2[:],
                                 bc_sb[0:B, BC_DB1 + 512:BC_DB1 + F])
            nc.vector.tensor_mul(z1_bt[0:B, :], tz[0:B, :], gp_bt[0:B, :])

            z1_t = wrk.tile([128, FO * B], F32)
            for fo in range(FO):
                tp = pss.tile([128, 32], F32, name="ps", tag="ps")
                nc.tensor.transpose(tp[:],
                                    z1_bt[0:32, fo * 128:(fo + 1) * 128], eye)
                nc.vector.tensor_copy(z1_t[:, fo * B:(fo + 1) * B], tp[:, 0:B])

            o5 = pss.tile([B, 512], F32, name="ps", tag="ps")
            o2 = pss.tile([B, 256], F32, name="ps", tag="ps")
            for k in range(KF):
                nc.tensor.matmul(o5[:], z1_t[:, k * B:(k + 1) * B],
                                 w2_sb[:, k * F:k * F + 512],
                                 start=(k == 0), stop=(k == KF - 1))
                nc.tensor.matmul(o2[:], z1_t[:, k * B:(k + 1) * B],
                                 w2_sb[:, k * F + 512:(k + 1) * F],
                                 start=(k == 0), stop=(k == KF - 1))
            out_bt = wrk.tile([32, F], F32)
            nc.vector.tensor_add(out_bt[0:B, 0:512], o5[:], v_bt[0:B, 0:512])
            nc.vector.tensor_add(out_bt[0:B, 512:F], o2[:], v_bt[0:B, 512:F])
            out2 = wrk.tile([32, F], F32)
            nc.vector.tensor_scalar(out2[0:B, :], out_bt[0:B, :], csel[:],
                                    None, OP.mult)
            nc.scalar.dma_start(OT.ap(), out2[0:B, :])

    nc.compile()
    return nc


def _get_nc():
    if "nc" not in _CACHE:
        _CACHE["nc"] = build()
    return _CACHE["nc"]


def _prep_in_maps(x, W1, b1, W2, b2, mW1, mb1, mW2, mb2, dW1, db1, dW2, db2):
    f32 = np.float32
    X = np.ascontiguousarray(np.asarray(x, f32).reshape(B, D))
    XT = np.ascontiguousarray(X.T)
    W1 = np.asarray(W1, f32)
    W2c = np.ascontiguousarray(np.asarray(W2, f32))
    mw1t = np.ascontiguousarray(np.asarray(mW1, f32).T)
    mw2t = np.ascontiguousarray(np.asarray(mW2, f32).T)
    b1 = np.asarray(b1, f32); b2 = np.asarray(b2, f32)
    mb1 = np.asarray(mb1, f32); mb2 = np.asarray(mb2, f32)
    dW1 = np.asarray(dW1, f32); db1 = np.asarray(db1, f32)
    dW2 = np.asarray(dW2, f32); db2 = np.asarray(db2, f32)

    in_maps = []
    for c in range(NCORES):
        bc = np.zeros((32, BC_W), f32)
        bc[0:B, BC_B1:BC_B1 + F] = b1[None, :]
        bc[0:B, BC_B2:BC_B2 + F] = b2[None, :]
        bc[0:B, BC_DB1:BC_DB1 + F] = db1[c][None, :]
        bc[0:B, BC_DB2:BC_DB2 + F] = db2[c][None, :]
        bc[0:B, BC_MB1:BC_MB1 + HID] = mb1[None, :]
        bc[0:B, BC_MB2:BC_MB2 + T] = mb2[None, :]
        bc[0:B, BC_SEL + c] = 1.0
        bc[0:32, BC_EYE:BC_EYE + 32] = np.eye(32, dtype=f32)
        in_maps.append({
            "xt": XT,
            "xts": np.ascontiguousarray(XT[c * DSH:(c + 1) * DSH, :]),
            "w1s": np.ascontiguousarray(W1[c * DSH:(c + 1) * DSH, :]),
            "w2": W2c,
            "dw1": np.ascontiguousarray(dW1[c]),
            "dw2": np.ascontiguousarray(dW2[c]),
            "mw1t": mw1t,
            "mw2t": mw2t,
            "bc": bc,
        })
    return in_maps


def run(inputs, trace=False, trace_cores=None, tmpdir=None):
    nc = _get_nc()
    in_maps = _prep_in_maps(**inputs)
    res = bass_utils.run_bass_kernel_spmd(
        nc, in_maps, core_ids=list(range(NCORES)), trace=trace,
        trace_cores=trace_cores, tmpdir=tmpdir)
    acc = res.results[0]["ft"].astype(np.float64)
    for c in range(NCORES):
        acc = acc + res.results[c]["ot"].astype(np.float64)
    return acc.astype(np.float32), res


def kernel(**inputs):
    out, _ = run(inputs, trace=False)
    return out


# revision 9
# speedup vs baseline: 2.1698x; 1.3820x over previous
"""Trainium2 Bass kernel for nn_MetaNetLinearizedModel.

Math (B=16, D=12288, F=768, HID=192, T=8):
    X = x.reshape(B, D)
    h1 = X @ W1 + b1                       [B, F]
    g  = gelu_tanh(h1); gp = gelu_tanh'(h1)
    feats = g @ W2 + b2                    [B, F]
    mh = relu(feats @ mW1.T + mb1)         [B, HID]
    coefs = mh @ mW2.T + mb2               [B, T]
    dh1   = sum_t coefs[:,t] * (X @ dW1[t] + db1[t])
    dout  = (gp * dh1) @ W2 + sum_t coefs[:,t] * (g @ dW2[t] + db2[t])
    out   = feats + dout

Sharding over T (8 task vectors -> 8 cores). Core c owns t=c:
    - h1 partial from a D-shard of W1 (D/8 rows), AllReduce -> full h1
    - U = X @ dW1[c]  (the big stream; bf16 halves the HBM traffic)
    - p_dout[c] = (gp * (U + db1[c])) @ W2 + (g @ dW2[c] + db2[c]),
      all scaled by coefs[:,c] at the end (the scale commutes)
    - host: out = feats + sum_c p_dout[c]

Orientation: outputs live in [B(=16, padded 32) partitions, F free] layout.
Big tensors (dW1, W1-shard, W2, dW2, mW1T) are MOVING operands (N=512/256),
small activation [d|f, b] tiles are stationary.
Precision: h1/feats/base path fp32 (dominates the output); delta paths
(U, V, Z2) bf16; meta-net fp32r.
"""
import sys

sys.path.insert(0, "/opt/trn_rl_repo")

import numpy as np
import ml_dtypes
import concourse.bass as bass
import concourse.bacc as bacc
import concourse.tile as tile
import concourse.mybir as mybir
from concourse import bass_utils

F32 = mybir.dt.float32
F32R = mybir.dt.float32r
BF16 = mybir.dt.bfloat16
AF = mybir.ActivationFunctionType
OP = mybir.AluOpType

B = 16
D = 3 * 64 * 64        # 12288
F = 768
HID = 192
T = 8
NCORES = 8
DSH = D // NCORES      # 1536
KD = D // 128          # 96 k-tiles over D
KJ = KD // 2           # 48 double-tiles
KSH = DSH // 128       # 12
FO = F // 128          # 6
KF = F // 128          # 6
GELU_C0 = float(np.sqrt(2.0 / np.pi))
GELU_A = 0.044715

# bc pack column offsets
BC_B1, BC_B2, BC_DB1, BC_DB2 = 0, F, 2 * F, 3 * F
BC_MB1 = 4 * F                 # width HID
BC_MB2 = BC_MB1 + HID          # width T
BC_SEL = BC_MB2 + T            # width T
BC_EYE = BC_SEL + T            # width 32
BC_W = BC_EYE + 32             # 3312

_CACHE = {}


def build():
    nc = bacc.Bacc("TRN2", target_bir_lowering=False, debug=False,
                   enable_asserts=False, num_devices=NCORES)

    XT = nc.dram_tensor("xt", [D, B], BF16, kind="ExternalInput")
    XTS = nc.dram_tensor("xts", [DSH, B], F32, kind="ExternalInput")
    W1S = nc.dram_tensor("w1s", [DSH, F], F32, kind="ExternalInput")
    W2 = nc.dram_tensor("w2", [F, F], F32, kind="ExternalInput")
    DW1 = nc.dram_tensor("dw1", [D, F], BF16, kind="ExternalInput")
    DW2 = nc.dram_tensor("dw2", [F, F], BF16, kind="ExternalInput")
    MW1T = nc.dram_tensor("mw1t", [F, HID], F32R, kind="ExternalInput")
    MW2T = nc.dram_tensor("mw2t", [HID, T], F32R, kind="ExternalInput")
    BC = nc.dram_tensor("bc", [32, BC_W], F32, kind="ExternalInput")
    OT = nc.dram_tensor("ot", [B, F], F32, kind="ExternalOutput")
    FT = nc.dram_tensor("ft", [B, F], F32, kind="ExternalOutput")

    with tile.TileContext(nc, num_cores=NCORES) as tc:
        with (
            tc.tile_pool(name="cst", bufs=1) as cst,
            tc.tile_pool(name="wrk", bufs=1) as wrk,
            tc.tile_pool(name="gtmp", bufs=4) as gtmp,
            tc.tile_pool(name="w1sp", bufs=4) as w1sp,
            tc.tile_pool(name="dw1p", bufs=12) as dw1p,
            tc.tile_pool(name="psu", bufs=1, space="PSUM") as psu,
            tc.tile_pool(name="pss", bufs=4, space="PSUM") as pss,
            tc.tile_pool(name="drm", bufs=1, space="DRAM") as drm,
        ):
            # ---- critical h1 path loads first, on the SYNC ring ----
            xts_sb = cst.tile([128, KSH * B], F32)
            nc.sync.dma_start(
                xts_sb[:].rearrange("p (n b) -> p n b", b=B),
                XTS.ap().rearrange("(n p) b -> p n b", p=128))

            # h1 partial (fp32), weights streamed on sync ring
            h5 = pss.tile([B, 512], F32, name="ps", tag="ps")
            h2 = pss.tile([B, 256], F32, name="ps", tag="ps")
            for k in range(KSH):
                wt = w1sp.tile([128, F], F32, name="w1t", tag="w1t")
                nc.sync.dma_start(wt[:], W1S.ap()[k * 128:(k + 1) * 128, :])
                nc.tensor.matmul(h5[:], xts_sb[:, k * B:(k + 1) * B],
                                 wt[:, 0:512],
                                 start=(k == 0), stop=(k == KSH - 1))
                nc.tensor.matmul(h2[:], xts_sb[:, k * B:(k + 1) * B],
                                 wt[:, 512:F],
                                 start=(k == 0), stop=(k == KSH - 1))
            h1p_sb = wrk.tile([32, F], F32)
            nc.vector.tensor_copy(h1p_sb[0:B, 0:512], h5[:])
            nc.vector.tensor_copy(h1p_sb[0:B, 512:F], h2[:])
            h1p_d = drm.tile([B, F], F32)
            h1ar_d = drm.tile([B, F], F32)
            nc.gpsimd.dma_start(h1p_d[:], h1p_sb[0:B, :])
            nc.gpsimd.collective_compute(
                "AllReduce", OP.add,
                replica_groups=[list(range(NCORES))],
                ins=[h1p_d[:]], outs=[h1ar_d[:]])
            h1r_sb = wrk.tile([32, F], F32)
            nc.gpsimd.dma_start(h1r_sb[0:B, :], h1ar_d[:])

            # ---- remaining loads: xt + dW1 stream on sync; rest on scalar ----
            xt_sb = cst.tile([128, KD * B], BF16)
            nc.sync.dma_start(
                xt_sb[:].rearrange("p (n b) -> p n b", b=B),
                XT.ap().rearrange("(n p) b -> p n b", p=128))

            bc_sb = cst.tile([32, BC_W], F32)
            nc.scalar.dma_start(bc_sb[:], BC.ap())
            w2_sb = cst.tile([128, KF * F], F32)
            for k in range(KF):
                nc.scalar.dma_start(
                    w2_sb[:, k * F:(k + 1) * F],
                    W2.ap()[k * 128:(k + 1) * 128, :])
            mw1t_sb = cst.tile([128, KF * HID], F32R)
            for k in range(KF):
                nc.scalar.dma_start(
                    mw1t_sb[:, k * HID:(k + 1) * HID],
                    MW1T.ap()[k * 128:(k + 1) * 128, :])
            mw2t_sb = cst.tile([128, 2 * T], F32R)
            nc.scalar.dma_start(mw2t_sb[:, 0:T], MW2T.ap()[0:128, :])
            nc.scalar.dma_start(mw2t_sb[0:HID - 128, T:2 * T],
                                MW2T.ap()[128:HID, :])
            dw2_sb = cst.tile([128, KF * F], BF16)
            for k in range(KF):
                nc.scalar.dma_start(
                    dw2_sb[:, k * F:(k + 1) * F],
                    DW2.ap()[k * 128:(k + 1) * 128, :])
            # bf16 copy of W2 for the (error-tolerant) Z2 path
            w2bf_sb = cst.tile([128, KF * F], BF16)
            nc.vector.tensor_copy(w2bf_sb[:], w2_sb[:])

            def bcs(col, w):
                return bc_sb[0:B, col:col + w]
            eye = bc_sb[0:32, BC_EYE:BC_EYE + 32]

            # ---- h1 = allreduce(h1p) + b1; gelu chain (rows 0:B) ----
            h1_bt = wrk.tile([32, F], F32)
            nc.vector.tensor_add(h1_bt[0:B, :], h1r_sb[0:B, :],
                                 bcs(BC_B1, F))

            def gt():
                return gtmp.tile([32, F], F32, name="gt", tag="gt")
            h1 = h1_bt[0:B, :]
            s_ = gt(); nc.vector.tensor_mul(s_[0:B, :], h1, h1)
            t1 = gt(); nc.vector.scalar_tensor_tensor(
                t1[0:B, :], s_[0:B, :], GELU_A, h1, OP.mult, OP.mult)
            up = gt(); nc.vector.tensor_add(up[0:B, :], h1, t1[0:B, :])
            t_sb = wrk.tile([32, F], F32)
            nc.scalar.activation(t_sb[0:B, :], up[0:B, :], AF.Tanh,
                                 scale=GELU_C0)
            tt = t_sb[0:B, :]
            t2 = gt(); nc.vector.tensor_mul(t2[0:B, :], tt, tt)
            q_ = gt(); nc.vector.tensor_scalar(q_[0:B, :], t2[0:B, :],
                                               -1.0, 1.0, OP.mult, OP.add)
            r_ = gt(); nc.vector.tensor_scalar(r_[0:B, :], s_[0:B, :],
                                               3.0 * GELU_A, 1.0,
                                               OP.mult, OP.add)
            m_ = gt(); nc.vector.tensor_mul(m_[0:B, :], q_[0:B, :], r_[0:B, :])
            n_ = gt(); nc.vector.tensor_mul(n_[0:B, :], m_[0:B, :], h1)
            th_sb = wrk.tile([32, F], F32)
            nc.vector.tensor_scalar(th_sb[0:B, :], tt, 0.5, 0.5,
                                    OP.mult, OP.add)
            gp_bt = wrk.tile([32, F], F32)
            nc.vector.scalar_tensor_tensor(
                gp_bt[0:B, :], n_[0:B, :], 0.5 * GELU_C0, th_sb[0:B, :],
                OP.mult, OP.add)
            g_bt = wrk.tile([32, F], F32)
            nc.vector.memset(g_bt[:], 0.0)
            nc.vector.tensor_mul(g_bt[0:B, :], th_sb[0:B, :], h1)

            # ---- transpose g -> [f, b] stationary tiles ----
            g_t = wrk.tile([128, FO * B], F32)
            g_tb = wrk.tile([128, FO * B], BF16)
            for fo in range(FO):
                tp = pss.tile([128, 32], F32, name="ps", tag="ps")
                nc.tensor.transpose(tp[:], g_bt[0:32, fo * 128:(fo + 1) * 128],
                                    eye)
                nc.vector.tensor_copy(g_t[:, fo * B:(fo + 1) * B], tp[:, 0:B])
                nc.vector.tensor_copy(g_tb[:, fo * B:(fo + 1) * B], tp[:, 0:B])

            # ---- feats = g @ W2 + b2 (fp32) ----
            f5 = pss.tile([B, 512], F32, name="ps", tag="ps")
            f2 = pss.tile([B, 256], F32, name="ps", tag="ps")
            for k in range(KF):
                nc.tensor.matmul(f5[:], g_t[:, k * B:(k + 1) * B],
                                 w2_sb[:, k * F:k * F + 512],
                                 start=(k == 0), stop=(k == KF - 1))
                nc.tensor.matmul(f2[:], g_t[:, k * B:(k + 1) * B],
                                 w2_sb[:, k * F + 512:(k + 1) * F],
                                 start=(k == 0), stop=(k == KF - 1))
            feats_bt = wrk.tile([32, F], F32)
            nc.vector.memset(feats_bt[:], 0.0)
            nc.vector.tensor_add(feats_bt[0:B, 0:512], f5[:], bcs(BC_B2, 512))
            nc.vector.tensor_add(feats_bt[0:B, 512:F], f2[:],
                                 bc_sb[0:B, BC_B2 + 512:BC_B2 + F])
            nc.scalar.dma_start(FT.ap(), feats_bt[0:B, :])

            feats_tr = wrk.tile([128, FO * B], F32R)
            for fo in range(FO):
                tp = pss.tile([128, 32], F32, name="ps", tag="ps")
                nc.tensor.transpose(tp[:],
                                    feats_bt[0:32, fo * 128:(fo + 1) * 128],
                                    eye)
                nc.vector.tensor_copy(feats_tr[:, fo * B:(fo + 1) * B],
                                      tp[:, 0:B])

            # ---- meta-net (fp32r) ----
            mps = pss.tile([B, HID], F32, name="ps", tag="ps")
            for k in range(KF):
                nc.tensor.matmul(mps[:], feats_tr[:, k * B:(k + 1) * B],
                                 mw1t_sb[:, k * HID:(k + 1) * HID],
                                 start=(k == 0), stop=(k == KF - 1))
            mh_bt = wrk.tile([32, HID], F32)
            nc.vector.memset(mh_bt[:], 0.0)
            mtmp = gtmp.tile([32, F], F32, name="gt", tag="gt")
            nc.vector.tensor_add(mtmp[0:B, 0:HID], mps[:], bcs(BC_MB1, HID))
            nc.vector.tensor_relu(mh_bt[0:B, :], mtmp[0:B, 0:HID])

            mh_tr = wrk.tile([128, 2 * B], F32R)
            tp = pss.tile([128, 32], F32, name="ps", tag="ps")
            nc.tensor.transpose(tp[:], mh_bt[0:32, 0:128], eye)
            nc.vector.tensor_copy(mh_tr[:, 0:B], tp[:, 0:B])
            tp = pss.tile([128, 32], F32, name="ps", tag="ps")
            nc.tensor.transpose(tp[0:HID - 128, :], mh_bt[0:32, 128:HID], eye)
            nc.vector.tensor_copy(mh_tr[0:HID - 128, B:2 * B],
                                  tp[0:HID - 128, 0:B])

            cps = pss.tile([B, T], F32, name="ps", tag="ps")
            nc.tensor.matmul(cps[:], mh_tr[:, 0:B], mw2t_sb[:, 0:T],
                             start=True, stop=False)
            nc.tensor.matmul(cps[:], mh_tr[0:HID - 128, B:2 * B],
                             mw2t_sb[0:HID - 128, T:2 * T],
                             start=False, stop=True)
            coefs_bt = wrk.tile([B, T], F32)
            nc.vector.tensor_add(coefs_bt[:], cps[:], bcs(BC_MB2, T))
            csel = wrk.tile([B, 1], F32)
            cjunk = wrk.tile([B, T], F32)
            nc.vector.tensor_mul(cjunk[:], coefs_bt[:], bcs(BC_SEL, T))
            nc.vector.reduce_sum(csel[:], cjunk[:], axis=mybir.AxisListType.X)

            # ---- V = g @ dW2 + db2 (bf16) ----
            v5 = pss.tile([B, 512], F32, name="ps", tag="ps")
            v2 = pss.tile([B, 256], F32, name="ps", tag="ps")
            for k in range(KF):
                nc.tensor.matmul(v5[:], g_tb[:, k * B:(k + 1) * B],
                                 dw2_sb[:, k * F:k * F + 512],
                                 start=(k == 0), stop=(k == KF - 1))
                nc.tensor.matmul(v2[:], g_tb[:, k * B:(k + 1) * B],
                                 dw2_sb[:, k * F + 512:(k + 1) * F],
                                 start=(k == 0), stop=(k == KF - 1))
            v_bt = wrk.tile([32, F], F32)
            nc.vector.tensor_add(v_bt[0:B, 0:512], v5[:], bcs(BC_DB2, 512))
            nc.vector.tensor_add(v_bt[0:B, 512:F], v2[:],
                                 bc_sb[0:B, BC_DB2 + 512:BC_DB2 + F])

            # ---- U = X @ dW1 (bf16 big stream, sync ring) ----
            u5 = psu.tile([B, 512], F32, name="u5")
            u2 = psu.tile([B, 256], F32, name="u2")
            dw1_r = DW1.ap().rearrange("(n two p) f -> n p two f", two=2,
                                       p=128)
            for j in range(KJ):
                dt_ = dw1p.tile([128, 2 * F], BF16, name="dw1t", tag="dw1t")
                nc.sync.dma_start(
                    dt_[:].rearrange("p (two f) -> p two f", two=2), dw1_r[j])
                for half in range(2):
                    d = 2 * j + half
                    nc.tensor.matmul(
                        u5[:], xt_sb[:, d * B:(d + 1) * B],
                        dt_[:, half * F:half * F + 512],
                        start=(d == 0), stop=(d == KD - 1),
                        skip_group_check=True)
                    nc.tensor.matmul(
                        u2[:], xt_sb[:, d * B:(d + 1) * B],
                        dt_[:, half * F + 512:(half + 1) * F],
                        start=(d == 0), stop=(d == KD - 1),
                        skip_group_check=True)

            # ---- tail ----
            z1_bt = wrk.tile([32, F], F32)
            nc.vector.memset(z1_bt[:], 0.0)
            tz = gtmp.tile([32, F], F32, name="gt", tag="gt")
            nc.vector.tensor_add(tz[0:B, 0:512], u5[:], bcs(BC_DB1, 512))
            nc.vector.tensor_add(tz[0:B, 512:F], u2[:],
                                 bc_sb[0:B, BC_DB1 + 512:BC_DB1 + F])
            nc.vector.tensor_mul(z1_bt[0:B, :], tz[0:B, :], gp_bt[0:B, :])

            z1_tb = wrk.tile([128, FO * B], BF16)
            for fo in range(FO):
                tp = pss.tile([128, 32], F32, name="ps", tag="ps")
                nc.tensor.transpose(tp[:],
                                    z1_bt[0:32, fo * 128:(fo + 1) * 128], eye)
                nc.vector.tensor_copy(z1_tb[:, fo * B:(fo + 1) * B],
                                      tp[:, 0:B])

            o5 = pss.tile([B, 512], F32, name="ps", tag="ps")
            o2 = pss.tile([B, 256], F32, name="ps", tag="ps")
            for k in range(KF):
                nc.tensor.matmul(o5[:], z1_tb[:, k * B:(k + 1) * B],
                                 w2bf_sb[:, k * F:k * F + 512],
                                 start=(k == 0), stop=(k == KF - 1))
                nc.tensor.matmul(o2[:], z1_tb[:, k * B:(k + 1) * B],
                                 w2bf_sb[:, k * F + 512:(k + 1) * F],
                                 start=(k == 0), stop=(k == KF - 1))
            out_bt = wrk.tile([32, F], F32)
            nc.vector.tensor_add(out_bt[0:B, 0:512], o5[:], v_bt[0:B, 0:512])
            nc.vector.tensor_add(out_bt[0:B, 512:F], o2[:], v_bt[0:B, 512:F])
            out2 = wrk.tile([32, F], F32)
            nc.vector.tensor_scalar(out2[0:B, :], out_bt[0:B, :], csel[:],
                                    None, OP.mult)
            nc.scalar.dma_start(OT.ap(), out2[0:B, :])

    nc.compile()
    return nc


def _get_nc():
    if "nc" not in _CACHE:
        _CACHE["nc"] = build()
    return _CACHE["nc"]


def _prep_in_maps(x, W1, b1, W2, b2, mW1, mb1, mW2, mb2, dW1, db1, dW2, db2):
    f32 = np.float32
    bf16 = ml_dtypes.bfloat16
    X = np.ascontiguousarray(np.asarray(x, f32).reshape(B, D))
    XT = np.ascontiguousarray(X.T)
    XTb = XT.astype(bf16)
    W1 = np.asarray(W1, f32)
    W2c = np.ascontiguousarray(np.asarray(W2, f32))
    mw1t = np.ascontiguousarray(np.asarray(mW1, f32).T)
    mw2t = np.ascontiguousarray(np.asarray(mW2, f32).T)
    b1 = np.asarray(b1, f32); b2 = np.asarray(b2, f32)
    mb1 = np.asarray(mb1, f32); mb2 = np.asarray(mb2, f32)
    dW1 = np.asarray(dW1, f32); db1 = np.asarray(db1, f32)
    dW2 = np.asarray(dW2, f32); db2 = np.asarray(db2, f32)

    in_maps = []
    for c in range(NCORES):
        bc = np.zeros((32, BC_W), f32)
        bc[0:B, BC_B1:BC_B1 + F] = b1[None, :]
        bc[0:B, BC_B2:BC_B2 + F] = b2[None, :]
        bc[0:B, BC_DB1:BC_DB1 + F] = db1[c][None, :]
        bc[0:B, BC_DB2:BC_DB2 + F] = db2[c][None, :]
        bc[0:B, BC_MB1:BC_MB1 + HID] = mb1[None, :]
        bc[0:B, BC_MB2:BC_MB2 + T] = mb2[None, :]
        bc[0:B, BC_SEL + c] = 1.0
        bc[0:32, BC_EYE:BC_EYE + 32] = np.eye(32, dtype=f32)
        in_maps.append({
            "xt": XTb,
            "xts": np.ascontiguousarray(XT[c * DSH:(c + 1) * DSH, :]),
            "w1s": np.ascontiguousarray(W1[c * DSH:(c + 1) * DSH, :]),
            "w2": W2c,
            "dw1": np.ascontiguousarray(dW1[c]).astype(bf16),
            "dw2": np.ascontiguousarray(dW2[c]).astype(bf16),
            "mw1t": mw1t,
            "mw2t": mw2t,
            "bc": bc,
        })
    return in_maps


def run(inputs, trace=False, trace_cores=None, tmpdir=None):
    nc = _get_nc()
    in_maps = _prep_in_maps(**inputs)
    res = bass_utils.run_bass_kernel_spmd(
        nc, in_maps, core_ids=list(range(NCORES)), trace=trace,
        trace_cores=trace_cores, tmpdir=tmpdir)
    acc = res.results[0]["ft"].astype(np.float64)
    for c in range(NCORES):
        acc = acc + res.results[c]["ot"].astype(np.float64)
    return acc.astype(np.float32), res


def kernel(**inputs):
    out, _ = run(inputs, trace=False)
    return out
